# revision 17
# baseline (speedup 1.0000x reference)
"""Trainium2 Bass kernel for masked cross-attention + 2H->H tanh projection.

Reference computation (per batch b):
    S = Q @ K^T                      [Lq, Lk]   (Q := `output`, K := `context`)
    S = where(mask, -inf, S)
    A = softmax(S, axis=-1)          -> output `attn`
    M = A @ K                        [Lq, H]
    out = tanh([M, Q] @ W^T + b)     -> output `out`

Strategy: data-parallel over batch (16 batches -> 8 cores x 2 each).
Host-side prep: Q^T / K^T (fp32r, pre-rounded) and K / W^T / b (bf16) are
laid out partition-major on the host so every tensor lands in SBUF with one
large contiguous DMA and no on-chip transposes of the inputs.

Per 128-row q-block (software-pipelined by one stage):
  - scores in fp32r on the PE (full-precision path for the attn output),
    mask added via a 1-row (ones x mask_bias) matmul into the same PSUM
    accumulation group, split into two half-rows so each PSUM half frees
    independently (online softmax with per-half maxima, reconciled after)
  - softmax: DVE reduce_max(negate) + ACT exp with accumulated row sums
  - A^T via PE transposes (bf16); mix and projection in bf16
Measured accuracy vs fp32 reference: attn ~8e-4, out ~2.5e-3 rel L2.
"""

import numpy as np

import concourse.bacc as bacc
import concourse.bass as bass
import concourse.mybir as mybir
from concourse.tile import TileContext
from concourse.bass_utils import run_bass_kernel_spmd
from concourse.masks import make_identity

f32 = mybir.dt.float32
f32r = mybir.dt.float32r
bf16 = mybir.dt.bfloat16
AX = mybir.AxisListType.X
AF = mybir.ActivationFunctionType

B, LQ, LK, H = 16, 2048, 2048, 1024
N_CORES = 8
BPC = B // N_CORES  # batches per core

_NEG = -float(2.0 ** 100)  # mask penalty; exactly representable at any width


def _round_fp32r(x):
    """Round fp32 mantissa to 12 stored bits (matches the on-chip fp32r
    rounding copy as measured on hardware)."""
    u = x.view(np.uint32).astype(np.uint64)
    u = (u + np.uint64(1 << 10)) & np.uint64(0xFFFFF800)
    return u.astype(np.uint32).view(np.float32)


def gen_kernel(bpc=BPC, lq=LQ, lk=LK, h=H):
    nqb = lq // 128        # q-blocks per batch
    nkc = lk // 128        # 128-row k-chunks
    nhc = h // 128         # 128-row h-chunks
    nsg = lk // 512        # 512-wide score groups (must be even)
    nog = h // 512         # 512-wide output groups
    ndc = 2 * h // 128     # contraction chunks for the projection
    assert nsg % 2 == 0 and nkc % 2 == 0 and nhc % 2 == 0

    nc = bacc.Bacc(None, target_bir_lowering=False)

    qtr_d = nc.dram_tensor("qtr", [bpc, nqb, 128, nhc * 128], f32r,
                           kind="ExternalInput")
    q16_d = nc.dram_tensor("q16", [bpc, nqb, 128, nhc * 128], bf16,
                           kind="ExternalInput")
    ktr_d = nc.dram_tensor("ktr", [bpc, 128, nhc * lk], f32r,
                           kind="ExternalInput")
    k16_d = nc.dram_tensor("k16", [bpc, 128, nkc * h], bf16,
                           kind="ExternalInput")
    wt_d = nc.dram_tensor("wt16", [128, ndc * h], bf16, kind="ExternalInput")
    mb_d = nc.dram_tensor("maskbias", [bpc, lk], f32r, kind="ExternalInput")
    b_d = nc.dram_tensor("bvec16", [1, h], bf16, kind="ExternalInput")
    attn_d = nc.dram_tensor("attn", [bpc, lq, lk], f32, kind="ExternalOutput")
    out_d = nc.dram_tensor("out", [bpc, lq, h], f32, kind="ExternalOutput")

    with TileContext(nc) as tc:
        with (
            tc.tile_pool(name="const", bufs=1) as const,
            tc.tile_pool(name="big", bufs=1) as big,
            tc.tile_pool(name="work", bufs=2) as work,
            tc.tile_pool(name="work3", bufs=3) as work3,
            tc.tile_pool(name="ea", bufs=2) as ea,
            tc.tile_pool(name="stats", bufs=4) as stats,
            tc.tile_pool(name="ps_s", bufs=2, space="PSUM") as ps_s,
            tc.tile_pool(name="ps_tp", bufs=2, space="PSUM") as ps_tp,
            tc.tile_pool(name="ps_mp", bufs=1, space="PSUM") as ps_mp,
        ):
            # ---- constants ----
            ident = const.tile([128, 128], f32)
            make_identity(nc, ident)
            ident16 = const.tile([128, 128], bf16)
            nc.vector.tensor_copy(ident16, ident)
            ones0 = const.tile([1, 128], f32)
            nc.vector.memset(ones0, 1.0)
            ones_r = const.tile([1, 128], f32r)
            nc.vector.tensor_copy(ones_r, ones0)
            ones16 = const.tile([1, 128], bf16)
            nc.vector.memset(ones16, 1.0)
            b16 = const.tile([1, h], bf16)
            nc.sync.dma_start(out=b16, in_=b_d[:, :])

            # ---- weights tile (loaded inside batch 0, after K) ----
            wt16 = big.tile([128, ndc, h], bf16)

            for bi in range(bpc):
                # per-batch K^T (fp32r, two half-DMAs so scores can start
                # after the first half) and K (bf16)
                mb_r = big.tile([1, lk], f32r, tag="mbr")
                nc.sync.dma_start(out=mb_r, in_=mb_d[bi:bi + 1, :])
                ktil = big.tile([128, nhc, lk], f32r, tag="ktil")
                k16 = big.tile([128, nkc, h], bf16, tag="k16")

                state = {}

                def dma_q(i):
                    qr = work.tile([128, nhc, 128], f32r, tag="qr")
                    nc.sync.dma_start(out=qr, in_=qtr_d[bi, i, :, :])
                    q16 = work3.tile([128, nhc, 128], bf16, tag="q16")
                    nc.sync.dma_start(out=q16, in_=q16_d[bi, i, :, :])
                    state[i] = (qr, q16)

                def stage_scores(i):
                    qs = slice(i * 128, (i + 1) * 128)
                    qr, q16 = state[i]
                    hg = nsg // 2
                    nm_h, sum_h, c_h = [], [], []
                    e_sb = ea.tile([128, lk], f32, tag="e")
                    for hx in range(2):
                        s_ps = ps_s.tile([128, lk // 2], f32, tag="sh")
                        for g in range(hg):
                            gg = hx * hg + g
                            for hc in range(nhc):
                                nc.tensor.matmul(
                                    s_ps[:, g * 512:(g + 1) * 512],
                                    qr[:, hc, :],
                                    ktil[:, hc, gg * 512:(gg + 1) * 512],
                                    start=(hc == 0),
                                    stop=False,
                                )
                            nc.tensor.matmul(
                                s_ps[:, g * 512:(g + 1) * 512],
                                ones_r[:, :],
                                mb_r[:, gg * 512:(gg + 1) * 512],
                                start=False,
                                stop=True,
                            )
                        nm = stats.tile([128, 1], f32, tag=f"negmx{hx}")
                        nc.vector.reduce_max(nm, s_ps, axis=AX, negate=True)
                        nm_h.append(nm)
                        # online softmax: exp against this half's own max so the
                        # PSUM half releases without waiting for the other half
                        sm = stats.tile([128, 1], f32, tag=f"sums{hx}")
                        nc.scalar.activation(
                            e_sb[:, hx * (lk // 2):(hx + 1) * (lk // 2)],
                            s_ps, AF.Exp, bias=nm, accum_out=sm,
                        )
                        sum_h.append(sm)
                    # reconcile: m = max(mA, mB); c_h = exp(m_h - m)
                    negmx = stats.tile([128, 1], f32, tag="negmx")
                    nc.vector.tensor_tensor(
                        negmx, nm_h[0], nm_h[1], op=mybir.AluOpType.min
                    )
                    for hx in range(2):
                        d = stats.tile([128, 1], f32, tag=f"d{hx}")
                        nc.vector.tensor_sub(d, negmx, nm_h[hx])
                        c = stats.tile([128, 1], f32, tag=f"c{hx}")
                        nc.scalar.activation(c, d, AF.Exp)
                        c_h.append(c)
                    z0 = stats.tile([128, 1], f32, tag="z0")
                    nc.vector.tensor_mul(z0, sum_h[0], c_h[0])
                    z = stats.tile([128, 1], f32, tag="z")
                    nc.vector.scalar_tensor_tensor(
                        out=z, in0=sum_h[1], scalar=c_h[1],
                        in1=z0, op0=mybir.AluOpType.mult,
                        op1=mybir.AluOpType.add,
                    )
                    rcp = stats.tile([128, 1], f32, tag="rcp")
                    nc.vector.reciprocal(rcp, z)
                    for hx in range(2):
                        r_h = stats.tile([128, 1], f32, tag=f"r{hx}")
                        nc.vector.tensor_mul(r_h, rcp, c_h[hx])
                        nc.vector.tensor_scalar_mul(
                            e_sb[:, hx * (lk // 2):(hx + 1) * (lk // 2)],
                            e_sb[:, hx * (lk // 2):(hx + 1) * (lk // 2)],
                            r_h,
                        )
                    nc.scalar.dma_start(out=attn_d[bi, qs, :], in_=e_sb)
                    a16 = ea.tile([128, lk], bf16, tag="a16")
                    nc.vector.tensor_copy(a16, e_sb)
                    return a16, q16

                def stage_tail(i, a16, q16):
                    qs = slice(i * 128, (i + 1) * 128)
                    at16 = work.tile([128, nkc, 128], bf16, tag="at16")
                    hk = nkc // 2
                    for hf in range(2):
                        at_ps = ps_tp.tile([128, hk, 128], bf16, tag="tph")
                        for j in range(hk):
                            kc = hf * hk + j
                            nc.tensor.transpose(
                                at_ps[:, j, :],
                                a16[:, kc * 128:(kc + 1) * 128],
                                ident16,
                            )
                        nc.vector.tensor_copy(
                            at16[:, hf * hk:(hf + 1) * hk, :], at_ps
                        )

                    m_ps = ps_mp.tile([128, h], f32, tag="mp")
                    for kc in range(nkc):
                        for g in range(nog):
                            nc.tensor.matmul(
                                m_ps[:, g * 512:(g + 1) * 512],
                                at16[:, kc, :],
                                k16[:, kc, g * 512:(g + 1) * 512],
                                start=(kc == 0),
                                stop=(kc == nkc - 1),
                            )
                    m16 = work.tile([128, h], bf16, tag="m16")
                    nc.scalar.copy(m16, m_ps)

                    mt16 = work.tile([128, nhc, 128], bf16, tag="mt16")
                    hn = nhc // 2
                    for hf in range(2):
                        mt_ps = ps_tp.tile([128, hn, 128], bf16, tag="tph")
                        for j in range(hn):
                            hc = hf * hn + j
                            nc.tensor.transpose(
                                mt_ps[:, j, :],
                                m16[:, hc * 128:(hc + 1) * 128],
                                ident16,
                            )
                        nc.vector.tensor_copy(
                            mt16[:, hf * hn:(hf + 1) * hn, :], mt_ps
                        )

                    p_ps = ps_mp.tile([128, h], f32, tag="mp")
                    for dc in range(ndc):
                        lhs = mt16[:, dc, :] if dc < nhc else q16[:, dc - nhc, :]
                        for g in range(nog):
                            nc.tensor.matmul(
                                p_ps[:, g * 512:(g + 1) * 512],
                                lhs,
                                wt16[:, dc, g * 512:(g + 1) * 512],
                                start=(dc == 0),
                                stop=False,
                            )
                    for g in range(nog):
                        nc.tensor.matmul(
                            p_ps[:, g * 512:(g + 1) * 512],
                            ones16[:, :],
                            b16[:, g * 512:(g + 1) * 512],
                            start=False,
                            stop=True,
                        )
                    o_sb = work.tile([128, h], f32, tag="o")
                    nc.scalar.activation(o_sb, p_ps, AF.Tanh)
                    nc.scalar.dma_start(out=out_d[bi, qs, :], in_=o_sb)

                dma_q(0)
                ktr_v = ktr_d[bi, :, :].rearrange("p (c k) -> p c k", c=nhc)
                for hx in range(2):
                    nc.sync.dma_start(
                        out=ktil[:, :, hx * (lk // 2):(hx + 1) * (lk // 2)],
                        in_=ktr_v[:, :, hx * (lk // 2):(hx + 1) * (lk // 2)],
                    )
                nc.sync.dma_start(out=k16, in_=k16_d[bi, :, :])
                if bi == 0:
                    nc.sync.dma_start(out=wt16, in_=wt_d[:, :])
                pend = None
                for i in range(nqb):
                    if i + 1 < nqb:
                        dma_q(i + 1)
                    a16, q16 = stage_scores(i)
                    if pend is not None:
                        stage_tail(*pend)
                    pend = (i, a16, q16)
                    state.pop(i - 1, None)
                stage_tail(*pend)

    nc.finalize()
    return nc


def prep_inputs(output, context, mask, W, b, bpc, n_cores):
    """Host-side layout prep. Returns per-core in_maps."""
    lq, h = output.shape[1], output.shape[2]
    lk = context.shape[1]
    nqb, nkc, nhc = lq // 128, lk // 128, h // 128
    ndc = 2 * h // 128

    output = np.asarray(output, dtype=np.float32)
    context = np.asarray(context, dtype=np.float32)
    W = np.asarray(W, dtype=np.float32)
    b = np.asarray(b, dtype=np.float32)
    bf = np.dtype("bfloat16")

    # Q^T, block-major partition-major: qtr[b,i,p,c*128+q] = Q[b,i*128+q,c*128+p]
    qt = output.reshape(-1, nqb, 128, nhc, 128)             # [B, i, q, c, p]
    qt = np.ascontiguousarray(qt.transpose(0, 1, 4, 3, 2))  # [B, i, p, c, q]
    qtr = _round_fp32r(qt).reshape(-1, nqb, 128, nhc * 128)
    q16 = qt.astype(bf).reshape(-1, nqb, 128, nhc * 128)

    # K^T: ktr[b, p, c*lk+k] = K[b, k, c*128+p]
    kt = context.reshape(-1, lk, nhc, 128)                  # [B, k, c, p]
    kt = np.ascontiguousarray(kt.transpose(0, 3, 2, 1))     # [B, p, c, k]
    ktr = _round_fp32r(kt).reshape(-1, 128, nhc * lk)

    # K natural bf16: k16[b, p, kc*h + hcol] = K[b, kc*128+p, hcol]
    kn = context.reshape(-1, nkc, 128, h)                   # [B, kc, p, h]
    kn = np.ascontiguousarray(kn.transpose(0, 2, 1, 3))     # [B, p, kc, h]
    k16n = kn.astype(bf).reshape(-1, 128, nkc * h)

    # W^T bf16: wt[p, dc*h + ho] = W[ho, dc*128+p]
    wt = W.reshape(h, ndc, 128)                             # [ho, dc, p]
    wt = np.ascontiguousarray(wt.transpose(2, 1, 0))        # [p, dc, ho]
    wt16 = wt.astype(bf).reshape(128, ndc * h)

    maskbias = np.where(np.asarray(mask), np.float32(_NEG), np.float32(0.0))
    maskbias = np.ascontiguousarray(maskbias.astype(np.float32))
    b16 = b.reshape(1, h).astype(bf)

    in_maps = []
    for c in range(n_cores):
        s = slice(c * bpc, (c + 1) * bpc)
        in_maps.append({
            "qtr": qtr[s], "q16": q16[s], "ktr": ktr[s], "k16": k16n[s],
            "wt16": wt16, "maskbias": maskbias[s], "bvec16": b16,
        })
    return in_maps


_CACHED_NC = None


def kernel(output, context, mask, W, b):
    """Full-input entry point: shards batch over 8 cores, returns (out, attn)."""
    global _CACHED_NC
    if _CACHED_NC is None:
        _CACHED_NC = gen_kernel()
    nc = _CACHED_NC
    in_maps = prep_inputs(output, context, mask, W, b, BPC, N_CORES)
    res = run_bass_kernel_spmd(nc, in_maps, core_ids=list(range(N_CORES)))
    out = np.concatenate([r["out"] for r in res.results], axis=0)
    attn = np.concatenate([r["attn"] for r in res.results], axis=0)
    return out, attn
